# revision 47
# baseline (speedup 1.0000x reference)
"""CFNO forward kernel for Trainium2 (8 NeuronCores, data-parallel over batch).

The reference computes, per 16x16 patch p (flattened to 256):
    fft = FFT_256(p) (ortho); fc = fft @ Wc^T + bc; y = Re(IFFT_16(fc)) (ortho)
    z = y @ conv_w^T + conv_b;  out = GroupNorm_8(z) * gamma + beta

Because p is real and every step before GroupNorm is linear, the whole chain
folds into one real matrix on the host:
    M2 = Re(F @ Wc^T @ G) @ conv_w^T   [256, 16]
    b2 = Re(bc @ G) @ conv_w^T + conv_b [16]
    z  = p @ M2 + b2
(F = symmetric 256-pt DFT matrix / sqrt(256); G = inverse 16-pt DFT / sqrt(16))

On-device per core (one batch image, x [2048, 2048], fp16 activations):
  - the whole fp16 image is SBUF-resident (64 KB/partition), so input tiles
    never recycle buffers and every input DMA is issued up-front, split
    across both HWDGE rings (sync + scalar) for parallel descriptor gen
  - host pre-permutes each image row from (w, s2) to (s2, w) order so the
    matmul moving operand is CONTIGUOUS per s2 slice -- a strided 16-bit
    rhs streams at ~0.42 cols/cycle vs 1 col/cycle @2.4 GHz contiguous
    (measured 917 vs 216 ns per 512-col matmul)
  - SBUF layout [128 part=(hblk, s1), rb, s2, w]; per row tile, 16
    PSUM-accumulating fp16 matmuls (one per patch-column offset s2, PE at
    2.4 GHz) with a block-diagonal lhsT so all 8 h-blocks share a matmul;
    dummy warm-up matmuls during the initial DMA window ramp the PE p-state
  - fused bias add via ScalarE on PSUM->SBUF copy; bn_stats for moments
    reads PSUM directly (var is shift-invariant; bias added to mean later)
  - one mask-matmul does the grouped cross-partition reduce AND broadcast;
    Rsqrt activation folds sqrt+reciprocal into one op
  - fused (z * A + B) normalize split across ScalarE/VectorE, fp16 output
    shipped on both rings (host upcasts to fp32)
"""

import numpy as np
from contextlib import ExitStack

CHUNK = 16
GROUPS = 8
EPS = 1e-5
B, C, H, W = 8, 1, 2048, 2048
D = 16
D_IN = CHUNK * CHUNK * C  # 256
HP = H // CHUNK  # 128 patch rows
WP = W // CHUNK  # 128 patch cols
P = 128
# row tiles per core, in 128-image-row blocks; small first tile gets the
# matmuls started early, small last tiles shrink the post-stream compute
# tail. ALL x tiles ride the sync HWDGE ring in consumption order (a single
# ring sustains ~429 GB/s; strict in-order delivery, ~1.22 us/rb, vs the
# PE's ~0.9 us/rb consumption); weights/consts/output ride the scalar ring
TILE_Q = [1, 3, 4, 3, 2, 2, 1]
RB = 16  # total 128-row blocks (2048 rows)
N_CORES = 8

_CACHED_NC = {}


def _build_nc(mm_dtype="float16"):
    import concourse.bass as bass
    import concourse.tile as tile
    from concourse import bacc, mybir

    f32 = mybir.dt.float32
    mmdt = getattr(mybir.dt, mm_dtype)
    is16 = mm_dtype in ("float16", "bfloat16")
    out_dt = mybir.dt.float16 if is16 else f32
    nc = bacc.Bacc("TRN2", target_bir_lowering=False, debug=False,
                   num_devices=N_CORES)

    # host-prepacked partition-major layout [p=(hblk,s1), rb, s2, w]: each
    # partition's data is contiguous in HBM -> big per-partition descriptors
    x = nc.dram_tensor("x", [P, RB * W], mmdt, kind="ExternalInput").ap()
    # host-packed [p, s2, m] so the SBUF load is contiguous per partition
    wl = nc.dram_tensor("wl", [P, CHUNK * P], mmdt, kind="ExternalInput").ap()
    # gmask [P, P] and consts [P, 4] (b2, gamma, beta, -gamma) in one DMA
    gmc = nc.dram_tensor("gmc", [P, P + 4], f32, kind="ExternalInput").ap()
    # [p=(hblk,e), rg, w] flattened (rg = hi//8); host reorders to [D, HP, WP]
    out = nc.dram_tensor("out", [P, RB * WP], out_dt, kind="ExternalOutput").ap()

    Ident = mybir.ActivationFunctionType.Identity
    Sqrt = mybir.ActivationFunctionType.Sqrt
    n_tiles = len(TILE_Q)
    assert sum(TILE_Q) == RB
    assert nc.vector.BN_STATS_FMAX >= max(TILE_Q) * WP

    with tile.TileContext(nc) as tc, ExitStack() as ctx:
        const_pool = ctx.enter_context(tc.tile_pool(name="const", bufs=1))
        xin = ctx.enter_context(tc.tile_pool(name="xin", bufs=1))
        zpool = ctx.enter_context(tc.tile_pool(name="z", bufs=1))
        psum = ctx.enter_context(tc.tile_pool(name="psum", bufs=1, space="PSUM"))
        psg = ctx.enter_context(tc.tile_pool(name="psg", bufs=1, space="PSUM"))

        # ---- all input DMAs issued up-front, split across both HWDGE rings
        # (sync=SP, scalar=ACT) so descriptor gen pipelines and nothing waits
        # on buffer recycling (everything is SBUF-resident).
        wtile = const_pool.tile([P, CHUNK, P], mmdt)
        nc.scalar.dma_start(out=wtile.rearrange("p s m -> p (s m)"), in_=wl)
        gmct = const_pool.tile([P, P + 4], f32)
        nc.scalar.dma_start(out=gmct, in_=gmc)
        gmt = gmct[:, :P]
        cvt = gmct[:, P:]

        xts = []
        rg0s = []
        rg = 0
        for ti, q in enumerate(TILE_Q):
            rg0s.append(rg)
            xt = xin.tile([P, q, W], mmdt, tag=f"xt{ti}", name=f"xt{ti}")
            nc.sync.dma_start(out=xt.rearrange("p q c -> p (q c)"),
                              in_=x[:, rg * W:(rg + q) * W])
            xts.append(xt)
            rg += q

        epst = const_pool.tile([P, 1], f32)
        nc.vector.memset(epst, EPS)
        # touch Sqrt early so its ACT table loads during the stream,
        # not in the stats critical chain
        warm = const_pool.tile([P, 1], f32)
        nc.scalar.activation(out=warm, in_=epst, func=Sqrt, bias=epst)
        # dummy matmuls while the first DMAs stream: ramp the PE out of its
        # cold p-state (0.65/1.2 GHz) so real matmuls run at 2.4 GHz.
        # the warmup PSUM bank is reused for the gmask matmul later.
        scr = const_pool.tile([P, P], mmdt)
        nc.vector.memset(scr, 0.0)
        pw = psg.tile([P, P], f32)
        for _ in range(20):
            nc.tensor.matmul(pw, lhsT=scr, rhs=scr, start=True, stop=True)

        # z kept in fp16: halves SBUF traffic and doubles DVE normalize rate;
        # bn_stats reads the fp32 PSUM directly so stats stay full precision
        zall = zpool.tile([P, RB * WP], out_dt)
        statsall = zpool.tile([P, n_tiles, nc.vector.BN_STATS_DIM], f32)

        for ti, q in enumerate(TILE_Q):
            xt = xts[ti]
            rg0 = rg0s[ti]
            # host pre-permuted row layout: (rb, s2, w) -> rhs contiguous in w
            xs = xt.rearrange("p q (s w) -> p q s w", w=WP)
            pt = psum.tile([P, q, WP], f32, tag=f"pt{ti}", name=f"pt{ti}")
            for s2 in range(CHUNK):
                nc.tensor.matmul(pt, lhsT=wtile[:, s2, :],
                                 rhs=xs[:, :, s2, :],
                                 start=(s2 == 0), stop=(s2 == CHUNK - 1))
            # z (+bias) -> SBUF; partition (hblk, e), free (q, w).
            zsl = zall[:, rg0 * WP:(rg0 + q) * WP]
            nc.scalar.activation(out=zsl,
                                 in_=pt.rearrange("p a b -> p (a b)"),
                                 func=Ident, bias=cvt[:, 0:1])
            nc.vector.bn_stats(out=statsall[:, ti],
                               in_=pt.rearrange("p a b -> p (a b)"))

        # Per-partition mean'/var over all 2048 elements (bias-less mean')
        mv = zpool.tile([P, 2], f32)
        nc.vector.bn_aggr(out=mv, in_=statsall)
        # rhs = (mean_p, E[x^2]_p) with mean = mean' + b2;
        # E2 = var + mean^2 in one fused DVE op
        me2 = zpool.tile([P, 2], f32)
        nc.vector.tensor_add(me2[:, 0:1], mv[:, 0:1], cvt[:, 0:1])
        nc.vector.scalar_tensor_tensor(
            out=me2[:, 1:2], in0=me2[:, 0:1], scalar=me2[:, 0:1],
            in1=mv[:, 1:2], op0=mybir.AluOpType.mult,
            op1=mybir.AluOpType.add)
        # Grouped cross-partition average + broadcast in one matmul:
        # gp[p'] = (1/16) * sum_{p in group(p')} me2[p]
        gp = pw[:, :2]
        nc.tensor.matmul(gp, lhsT=gmt, rhs=me2, start=True, stop=True)
        gsb = zpool.tile([P, 2], f32)
        nc.vector.tensor_copy(gsb, gp)
        gmean = gsb[:, 0:1]
        # negvar = mean^2 - E2;  sd = sqrt(-negvar + eps); A = gamma/sd
        negvar = zpool.tile([P, 1], f32)
        nc.vector.scalar_tensor_tensor(
            out=negvar, in0=gmean, scalar=gmean, in1=gsb[:, 1:2],
            op0=mybir.AluOpType.mult, op1=mybir.AluOpType.subtract)
        sd = zpool.tile([P, 1], f32)
        nc.scalar.activation(out=sd, in_=negvar, func=Sqrt, bias=epst,
                             scale=-1.0)
        rs = zpool.tile([P, 1], f32)
        nc.vector.reciprocal(rs, sd)
        # out = z*A + Bp with A = gamma/sd, Bp = beta - mean*A
        A = zpool.tile([P, 1], f32)
        nc.vector.tensor_mul(A, rs, cvt[:, 1:2])
        negA = zpool.tile([P, 1], f32)
        nc.vector.tensor_mul(negA, rs, cvt[:, 3:4])
        Bp = zpool.tile([P, 1], f32)
        nc.vector.scalar_tensor_tensor(
            out=Bp, in0=gmean, scalar=negA, in1=cvt[:, 2:3],
            op0=mybir.AluOpType.mult, op1=mybir.AluOpType.add)

        # normalize: ACT takes a small contiguous head (it is ~2-3x slower
        # per element than DVE), DVE the rest in one op; two output chunks
        # keep the per-partition DMA descriptors big (the 0.5 MB output is
        # fixed-cost dominated, so fewer/bigger DMAs win over early ship)
        onorm = zpool.tile([P, RB * WP], out_dt)
        c0 = 4 * WP
        nc.vector.tensor_scalar(out=onorm[:, c0:], in0=zall[:, c0:],
                                scalar1=A, scalar2=Bp,
                                op0=mybir.AluOpType.mult,
                                op1=mybir.AluOpType.add)
        nc.sync.dma_start(out=out[:, c0:], in_=onorm[:, c0:])
        nc.scalar.activation(out=onorm[:, :c0], in_=zall[:, :c0],
                             func=Ident, scale=A, bias=Bp)
        nc.scalar.dma_start(out=out[:, :c0], in_=onorm[:, :c0])

    nc.compile()
    return nc


def _host_weights(fc_wr, fc_wi, fc_br, fc_bi, conv_w, conv_b, gamma, beta):
    fc_wr = np.asarray(fc_wr, np.float64)
    fc_wi = np.asarray(fc_wi, np.float64)
    fc_br = np.asarray(fc_br, np.float64)
    fc_bi = np.asarray(fc_bi, np.float64)
    conv_w = np.asarray(conv_w, np.float64)
    conv_b = np.asarray(conv_b, np.float64)
    gamma = np.asarray(gamma, np.float64)
    beta = np.asarray(beta, np.float64)

    j = np.arange(D_IN)
    F = np.exp(-2j * np.pi * np.outer(j, j) / D_IN) / np.sqrt(D_IN)
    d = np.arange(D)
    G = np.exp(2j * np.pi * np.outer(d, d) / D) / np.sqrt(D)
    Wc = fc_wr + 1j * fc_wi
    bc = fc_br + 1j * fc_bi
    M2 = (np.real(F @ Wc.T @ G) @ conv_w.T).astype(np.float32)  # [256, 16]
    b2 = (np.real(bc @ G) @ conv_w.T + conv_b).astype(np.float32)  # [16]

    # Block-diagonal lhsT: wl[hblk*16+s1, s2, hblk*16+e] = M2[s1*16+s2, e],
    # packed [p, s2*128+m] for a contiguous per-partition SBUF load.
    wl = np.zeros((CHUNK, P, P), np.float32)  # [s2, p, m]
    blk = M2.reshape(CHUNK, CHUNK, D).transpose(1, 0, 2)  # [s2, s1, e]
    for hb in range(8):
        wl[:, hb * 16:hb * 16 + 16, hb * 16:hb * 16 + 16] = blk
    wl = np.ascontiguousarray(wl.transpose(1, 0, 2).reshape(P, CHUNK * P))

    # Group-average + broadcast mask; rhs carries raw (mean, E2) per
    # partition; each group spans 16 partitions -> scale 1/16
    pidx = np.arange(P)
    grp = (pidx % D) // (D // GROUPS)
    gmask = (grp[:, None] == grp[None, :]).astype(np.float32) / 16.0

    e = pidx % D
    g32 = gamma.astype(np.float32)[e]
    consts = np.stack([b2[e], g32, beta.astype(np.float32)[e], -g32],
                      axis=1)  # [128, 4]
    gmc = np.ascontiguousarray(
        np.concatenate([gmask, consts], axis=1))  # [128, 132]
    return wl, gmc


def kernel(x, fc_wr, fc_wi, fc_br, fc_bi, conv_w, conv_b, gamma, beta,
           _return_results=False, _trace=False, _mm_dtype="float16"):
    from concourse.bass_utils import run_bass_kernel_spmd

    if _mm_dtype not in _CACHED_NC:
        _CACHED_NC[_mm_dtype] = _build_nc(_mm_dtype)
    nc = _CACHED_NC[_mm_dtype]
    is16 = _mm_dtype in ("float16", "bfloat16")
    np_mm = {"float16": np.float16, "bfloat16": np.float32}.get(
        _mm_dtype, np.float32)

    wl, gmc = _host_weights(fc_wr, fc_wi, fc_br, fc_bi,
                            conv_w, conv_b, gamma, beta)
    # cast to fp16 and prepack to the device layout [p=(hblk,s1), rb, s2, w]:
    # per-partition contiguous in HBM, matmul moving operand contiguous in w
    x = np.asarray(x, np.float32).reshape(B, H, W).astype(np_mm)
    x = np.ascontiguousarray(
        x.reshape(B, RB, 8, CHUNK, WP, CHUNK)
        .transpose(0, 2, 3, 1, 5, 4).reshape(B, P, RB * W))
    wl = wl.astype(np_mm)
    in_maps = [{"x": x[b], "wl": wl, "gmc": gmc} for b in range(N_CORES)]
    res = run_bass_kernel_spmd(nc, in_maps, list(range(N_CORES)),
                               trace=_trace)
    # device layout [p=(hblk,e), rg, w] -> [D, HP, WP], hi = rg*8 + hblk
    out = np.stack(
        [res.results[b]["out"].reshape(8, D, RB, WP)
         .transpose(1, 2, 0, 3).reshape(D, HP, WP)
         for b in range(N_CORES)], axis=0).astype(np.float32)
    if _return_results:
        return out, res
    return out


# revision 48
# speedup vs baseline: 1.1719x; 1.1719x over previous
"""CFNO forward kernel for Trainium2 (8 NeuronCores, data-parallel over batch).

The reference computes, per 16x16 patch p (flattened to 256):
    fft = FFT_256(p) (ortho); fc = fft @ Wc^T + bc; y = Re(IFFT_16(fc)) (ortho)
    z = y @ conv_w^T + conv_b;  out = GroupNorm_8(z) * gamma + beta

Because p is real and every step before GroupNorm is linear, the whole chain
folds into one real matrix on the host:
    M2 = Re(F @ Wc^T @ G) @ conv_w^T   [256, 16]
    b2 = Re(bc @ G) @ conv_w^T + conv_b [16]
    z  = p @ M2 + b2
(F = symmetric 256-pt DFT matrix / sqrt(256); G = inverse 16-pt DFT / sqrt(16))

On-device per core (one batch image, x [2048, 2048], fp16 activations):
  - the whole fp16 image is SBUF-resident (64 KB/partition), so input tiles
    never recycle buffers and every input DMA is issued up-front, split
    across both HWDGE rings (sync + scalar) for parallel descriptor gen
  - host pre-permutes each image row from (w, s2) to (s2, w) order so the
    matmul moving operand is CONTIGUOUS per s2 slice -- a strided 16-bit
    rhs streams at ~0.42 cols/cycle vs 1 col/cycle @2.4 GHz contiguous
    (measured 917 vs 216 ns per 512-col matmul)
  - SBUF layout [128 part=(hblk, s1), rb, s2, w]; per row tile, 16
    PSUM-accumulating fp16 matmuls (one per patch-column offset s2, PE at
    2.4 GHz) with a block-diagonal lhsT so all 8 h-blocks share a matmul;
    dummy warm-up matmuls during the initial DMA window ramp the PE p-state
  - fused bias add via ScalarE on PSUM->SBUF copy; bn_stats for moments
    reads PSUM directly (var is shift-invariant; bias added to mean later)
  - one mask-matmul does the grouped cross-partition reduce AND broadcast;
    Rsqrt activation folds sqrt+reciprocal into one op
  - fused (z * A + B) normalize split across ScalarE/VectorE, fp16 output
    shipped on both rings (host upcasts to fp32)
"""

import numpy as np
from contextlib import ExitStack

CHUNK = 16
GROUPS = 8
EPS = 1e-5
B, C, H, W = 8, 1, 2048, 2048
D = 16
D_IN = CHUNK * CHUNK * C  # 256
HP = H // CHUNK  # 128 patch rows
WP = W // CHUNK  # 128 patch cols
P = 128
# row tiles per core, in 128-image-row blocks; small first tile gets the
# matmuls started early, small last tiles shrink the post-stream compute
# tail. ALL x tiles ride the sync HWDGE ring in consumption order (a single
# ring sustains ~429 GB/s; strict in-order delivery, ~1.22 us/rb, vs the
# PE's ~0.9 us/rb consumption); weights/consts/output ride the scalar ring
TILE_Q = [1, 3, 4, 3, 2, 2, 1]
RB = 16  # total 128-row blocks (2048 rows)
N_CORES = 8

_CACHED_NC = {}


def _build_nc(mm_dtype="float16"):
    import concourse.bass as bass
    import concourse.tile as tile
    from concourse import bacc, mybir

    f32 = mybir.dt.float32
    mmdt = getattr(mybir.dt, mm_dtype)
    is16 = mm_dtype in ("float16", "bfloat16")
    out_dt = mybir.dt.float16 if is16 else f32
    nc = bacc.Bacc("TRN2", target_bir_lowering=False, debug=False,
                   num_devices=N_CORES)

    # host-prepacked partition-major layout [p=(hblk,s1), rb, s2, w]: each
    # partition's data is contiguous in HBM -> big per-partition descriptors
    x = nc.dram_tensor("x", [P, RB * W], mmdt, kind="ExternalInput").ap()
    # host-packed [p, s2, m] so the SBUF load is contiguous per partition
    wl = nc.dram_tensor("wl", [P, CHUNK * P], mmdt, kind="ExternalInput").ap()
    # gmask [P, P] and consts [P, 4] (b2, gamma, beta, -gamma) in one DMA
    gmc = nc.dram_tensor("gmc", [P, P + 4], f32, kind="ExternalInput").ap()
    # [p=(hblk,e), rg, w] flattened (rg = hi//8); host reorders to [D, HP, WP]
    out = nc.dram_tensor("out", [P, RB * WP], out_dt, kind="ExternalOutput").ap()

    Ident = mybir.ActivationFunctionType.Identity
    Sqrt = mybir.ActivationFunctionType.Sqrt
    n_tiles = len(TILE_Q)
    assert sum(TILE_Q) == RB
    assert nc.vector.BN_STATS_FMAX >= max(TILE_Q) * WP

    with tile.TileContext(nc) as tc, ExitStack() as ctx:
        const_pool = ctx.enter_context(tc.tile_pool(name="const", bufs=1))
        xin = ctx.enter_context(tc.tile_pool(name="xin", bufs=1))
        zpool = ctx.enter_context(tc.tile_pool(name="z", bufs=1))
        psum = ctx.enter_context(tc.tile_pool(name="psum", bufs=1, space="PSUM"))
        psg = ctx.enter_context(tc.tile_pool(name="psg", bufs=1, space="PSUM"))

        # ---- all input DMAs issued up-front, split across both HWDGE rings
        # (sync=SP, scalar=ACT) so descriptor gen pipelines and nothing waits
        # on buffer recycling (everything is SBUF-resident).
        wtile = const_pool.tile([P, CHUNK, P], mmdt)
        nc.scalar.dma_start(out=wtile.rearrange("p s m -> p (s m)"), in_=wl)
        gmct = const_pool.tile([P, P + 4], f32)
        nc.scalar.dma_start(out=gmct, in_=gmc)
        gmt = gmct[:, :P]
        cvt = gmct[:, P:]

        xts = []
        rg0s = []
        rg = 0
        for ti, q in enumerate(TILE_Q):
            rg0s.append(rg)
            xt = xin.tile([P, q, W], mmdt, tag=f"xt{ti}", name=f"xt{ti}")
            nc.sync.dma_start(out=xt.rearrange("p q c -> p (q c)"),
                              in_=x[:, rg * W:(rg + q) * W])
            xts.append(xt)
            rg += q

        epst = const_pool.tile([P, 1], f32)
        nc.vector.memset(epst, EPS)
        # touch Sqrt early so its ACT table loads during the stream,
        # not in the stats critical chain
        warm = const_pool.tile([P, 1], f32)
        nc.scalar.activation(out=warm, in_=epst, func=Sqrt, bias=epst)
        # dummy matmuls while the first DMAs stream: ramp the PE out of its
        # cold p-state (0.65/1.2 GHz) so real matmuls run at 2.4 GHz.
        # the warmup PSUM bank is reused for the gmask matmul later.
        scr = const_pool.tile([P, P], mmdt)
        nc.vector.memset(scr, 0.0)
        pw = psg.tile([P, P], f32)
        for _ in range(20):
            nc.tensor.matmul(pw, lhsT=scr, rhs=scr, start=True, stop=True)

        # z kept in fp16: halves SBUF traffic and doubles DVE normalize rate;
        # bn_stats reads the fp32 PSUM directly so stats stay full precision
        zall = zpool.tile([P, RB * WP], out_dt)
        statsall = zpool.tile([P, n_tiles, nc.vector.BN_STATS_DIM], f32)

        for ti, q in enumerate(TILE_Q):
            xt = xts[ti]
            rg0 = rg0s[ti]
            # host pre-permuted row layout: (rb, s2, w) -> rhs contiguous in w
            xs = xt.rearrange("p q (s w) -> p q s w", w=WP)
            pt = psum.tile([P, q, WP], f32, tag=f"pt{ti}", name=f"pt{ti}")
            for s2 in range(CHUNK):
                nc.tensor.matmul(pt, lhsT=wtile[:, s2, :],
                                 rhs=xs[:, :, s2, :],
                                 start=(s2 == 0), stop=(s2 == CHUNK - 1))
            # z (+bias) -> SBUF; partition (hblk, e), free (q, w).
            zsl = zall[:, rg0 * WP:(rg0 + q) * WP]
            nc.scalar.activation(out=zsl,
                                 in_=pt.rearrange("p a b -> p (a b)"),
                                 func=Ident, bias=cvt[:, 0:1])
            nc.vector.bn_stats(out=statsall[:, ti],
                               in_=pt.rearrange("p a b -> p (a b)"))

        # Per-partition mean'/var over all 2048 elements (bias-less mean')
        mv = zpool.tile([P, 2], f32)
        nc.vector.bn_aggr(out=mv, in_=statsall)
        # rhs = (mean_p, E[x^2]_p) with mean = mean' + b2;
        # E2 = var + mean^2 in one fused DVE op
        me2 = zpool.tile([P, 2], f32)
        nc.vector.tensor_add(me2[:, 0:1], mv[:, 0:1], cvt[:, 0:1])
        nc.vector.scalar_tensor_tensor(
            out=me2[:, 1:2], in0=me2[:, 0:1], scalar=me2[:, 0:1],
            in1=mv[:, 1:2], op0=mybir.AluOpType.mult,
            op1=mybir.AluOpType.add)
        # Grouped cross-partition average + broadcast in one matmul:
        # gp[p'] = (1/16) * sum_{p in group(p')} me2[p]
        gp = pw[:, :2]
        nc.tensor.matmul(gp, lhsT=gmt, rhs=me2, start=True, stop=True)
        gsb = zpool.tile([P, 2], f32)
        nc.vector.tensor_copy(gsb, gp)
        gmean = gsb[:, 0:1]
        # negvar = mean^2 - E2;  sd = sqrt(-negvar + eps); A = gamma/sd
        negvar = zpool.tile([P, 1], f32)
        nc.vector.scalar_tensor_tensor(
            out=negvar, in0=gmean, scalar=gmean, in1=gsb[:, 1:2],
            op0=mybir.AluOpType.mult, op1=mybir.AluOpType.subtract)
        sd = zpool.tile([P, 1], f32)
        nc.scalar.activation(out=sd, in_=negvar, func=Sqrt, bias=epst,
                             scale=-1.0)
        rs = zpool.tile([P, 1], f32)
        nc.vector.reciprocal(rs, sd)
        # out = z*A + Bp with A = gamma/sd, Bp = beta - mean*A
        A = zpool.tile([P, 1], f32)
        nc.vector.tensor_mul(A, rs, cvt[:, 1:2])
        negA = zpool.tile([P, 1], f32)
        nc.vector.tensor_mul(negA, rs, cvt[:, 3:4])
        Bp = zpool.tile([P, 1], f32)
        nc.vector.scalar_tensor_tensor(
            out=Bp, in0=gmean, scalar=negA, in1=cvt[:, 2:3],
            op0=mybir.AluOpType.mult, op1=mybir.AluOpType.add)

        # normalize: ACT takes a small contiguous head (it is ~2-3x slower
        # per element than DVE), DVE the rest in three chunks with a tiny
        # final one; each chunk ships on a ring as soon as it is normalized
        onorm = zpool.tile([P, RB * WP], out_dt)
        cuts = [(4 * WP, 12 * WP, nc.vector, nc.sync),
                (0, 4 * WP, nc.scalar, nc.scalar),
                (12 * WP, 15 * WP, nc.vector, nc.sync),
                (15 * WP, RB * WP, nc.vector, nc.scalar)]
        for lo, hi, ceng, deng in cuts:
            if ceng is nc.scalar:
                nc.scalar.activation(out=onorm[:, lo:hi], in_=zall[:, lo:hi],
                                     func=Ident, scale=A, bias=Bp)
            else:
                nc.vector.tensor_scalar(out=onorm[:, lo:hi],
                                        in0=zall[:, lo:hi],
                                        scalar1=A, scalar2=Bp,
                                        op0=mybir.AluOpType.mult,
                                        op1=mybir.AluOpType.add)
            deng.dma_start(out=out[:, lo:hi], in_=onorm[:, lo:hi])

    nc.compile()
    return nc


def _host_weights(fc_wr, fc_wi, fc_br, fc_bi, conv_w, conv_b, gamma, beta):
    fc_wr = np.asarray(fc_wr, np.float64)
    fc_wi = np.asarray(fc_wi, np.float64)
    fc_br = np.asarray(fc_br, np.float64)
    fc_bi = np.asarray(fc_bi, np.float64)
    conv_w = np.asarray(conv_w, np.float64)
    conv_b = np.asarray(conv_b, np.float64)
    gamma = np.asarray(gamma, np.float64)
    beta = np.asarray(beta, np.float64)

    j = np.arange(D_IN)
    F = np.exp(-2j * np.pi * np.outer(j, j) / D_IN) / np.sqrt(D_IN)
    d = np.arange(D)
    G = np.exp(2j * np.pi * np.outer(d, d) / D) / np.sqrt(D)
    Wc = fc_wr + 1j * fc_wi
    bc = fc_br + 1j * fc_bi
    M2 = (np.real(F @ Wc.T @ G) @ conv_w.T).astype(np.float32)  # [256, 16]
    b2 = (np.real(bc @ G) @ conv_w.T + conv_b).astype(np.float32)  # [16]

    # Block-diagonal lhsT: wl[hblk*16+s1, s2, hblk*16+e] = M2[s1*16+s2, e],
    # packed [p, s2*128+m] for a contiguous per-partition SBUF load.
    wl = np.zeros((CHUNK, P, P), np.float32)  # [s2, p, m]
    blk = M2.reshape(CHUNK, CHUNK, D).transpose(1, 0, 2)  # [s2, s1, e]
    for hb in range(8):
        wl[:, hb * 16:hb * 16 + 16, hb * 16:hb * 16 + 16] = blk
    wl = np.ascontiguousarray(wl.transpose(1, 0, 2).reshape(P, CHUNK * P))

    # Group-average + broadcast mask; rhs carries raw (mean, E2) per
    # partition; each group spans 16 partitions -> scale 1/16
    pidx = np.arange(P)
    grp = (pidx % D) // (D // GROUPS)
    gmask = (grp[:, None] == grp[None, :]).astype(np.float32) / 16.0

    e = pidx % D
    g32 = gamma.astype(np.float32)[e]
    consts = np.stack([b2[e], g32, beta.astype(np.float32)[e], -g32],
                      axis=1)  # [128, 4]
    gmc = np.ascontiguousarray(
        np.concatenate([gmask, consts], axis=1))  # [128, 132]
    return wl, gmc


def kernel(x, fc_wr, fc_wi, fc_br, fc_bi, conv_w, conv_b, gamma, beta,
           _return_results=False, _trace=False, _mm_dtype="float16"):
    from concourse.bass_utils import run_bass_kernel_spmd

    if _mm_dtype not in _CACHED_NC:
        _CACHED_NC[_mm_dtype] = _build_nc(_mm_dtype)
    nc = _CACHED_NC[_mm_dtype]
    is16 = _mm_dtype in ("float16", "bfloat16")
    np_mm = {"float16": np.float16, "bfloat16": np.float32}.get(
        _mm_dtype, np.float32)

    wl, gmc = _host_weights(fc_wr, fc_wi, fc_br, fc_bi,
                            conv_w, conv_b, gamma, beta)
    # cast to fp16 and prepack to the device layout [p=(hblk,s1), rb, s2, w]:
    # per-partition contiguous in HBM, matmul moving operand contiguous in w
    x = np.asarray(x, np.float32).reshape(B, H, W).astype(np_mm)
    x = np.ascontiguousarray(
        x.reshape(B, RB, 8, CHUNK, WP, CHUNK)
        .transpose(0, 2, 3, 1, 5, 4).reshape(B, P, RB * W))
    wl = wl.astype(np_mm)
    in_maps = [{"x": x[b], "wl": wl, "gmc": gmc} for b in range(N_CORES)]
    res = run_bass_kernel_spmd(nc, in_maps, list(range(N_CORES)),
                               trace=_trace)
    # device layout [p=(hblk,e), rg, w] -> [D, HP, WP], hi = rg*8 + hblk
    out = np.stack(
        [res.results[b]["out"].reshape(8, D, RB, WP)
         .transpose(1, 2, 0, 3).reshape(D, HP, WP)
         for b in range(N_CORES)], axis=0).astype(np.float32)
    if _return_results:
        return out, res
    return out
